# revision 1
# baseline (speedup 1.0000x reference)
"""Trainium2 Bass kernel for nn_Attention_69861938037658.

Computation per batch b (B=4096, S=200, H=128):
    proj  = X_b @ W1.T + (l_b @ W2.T)        # [S,H]
    hid   = tanh(proj)
    sc    = hid @ W3_w.T                      # [S]
    sc    = where(mask, -1e9, sc)
    attn  = softmax(sc)
    out_b = attn @ X_b                        # [H]

Sharding: pure data parallel, 512 batches per core on 8 cores.
"""

import sys
import numpy as np

if "/opt/trn_rl_repo" not in sys.path:
    sys.path.insert(0, "/opt/trn_rl_repo")

B, S, H = 4096, 200, 128
NCORES = 8
BC = B // NCORES          # 512 batches per core
BB = 64                   # batches per block
NBLK = BC // BB           # 8 blocks
NEG = -1.0e9

_cache = {}


def _build(nblk=NBLK):
    import concourse.bacc as bacc
    import concourse.tile as tile
    from concourse import mybir
    from concourse.masks import make_identity
    from contextlib import ExitStack

    f32 = mybir.dt.float32
    f32r = mybir.dt.float32r
    u8 = mybir.dt.uint8
    Tanh = mybir.ActivationFunctionType.Tanh
    Exp = mybir.ActivationFunctionType.Exp

    nc = bacc.Bacc("TRN2", target_bir_lowering=False, debug=False)
    bcp = nblk * BB  # batches this core actually processes

    x = nc.dram_tensor("x", [BC, S, H], f32, kind="ExternalInput")
    l = nc.dram_tensor("l", [BC, H], f32, kind="ExternalInput")
    m = nc.dram_tensor("m", [BC, S], u8, kind="ExternalInput")
    w1 = nc.dram_tensor("w1", [H, H], f32, kind="ExternalInput")
    w2 = nc.dram_tensor("w2", [H, H], f32, kind="ExternalInput")
    w3 = nc.dram_tensor("w3", [1, H], f32, kind="ExternalInput")
    out = nc.dram_tensor("out", [BC, H], f32, kind="ExternalOutput")

    def r(ap):
        return ap.bitcast(f32r)

    with tile.TileContext(nc) as tc, ExitStack() as ctx:
        singles = ctx.enter_context(tc.tile_pool(name="singles", bufs=1))
        xa_p = ctx.enter_context(tc.tile_pool(name="xa", bufs=2))
        xb_p = ctx.enter_context(tc.tile_pool(name="xb", bufs=2))
        xt_p = ctx.enter_context(tc.tile_pool(name="xt", bufs=4))
        hid_p = ctx.enter_context(tc.tile_pool(name="hid", bufs=4))
        stage_p = ctx.enter_context(tc.tile_pool(name="stage", bufs=4))
        sc_p = ctx.enter_context(tc.tile_pool(name="sc", bufs=2))
        small_p = ctx.enter_context(tc.tile_pool(name="small", bufs=3))
        o_p = ctx.enter_context(tc.tile_pool(name="o", bufs=2))
        xtps_p = ctx.enter_context(tc.tile_pool(name="xtps", bufs=2, space="PSUM"))
        pjps_p = ctx.enter_context(tc.tile_pool(name="pjps", bufs=2, space="PSUM"))
        scps_p = ctx.enter_context(tc.tile_pool(name="scps", bufs=2, space="PSUM"))
        mips_p = ctx.enter_context(tc.tile_pool(name="mips", bufs=1, space="PSUM"))
        ops_p = ctx.enter_context(tc.tile_pool(name="ops", bufs=1, space="PSUM"))

        # ---- constants / weights ----
        ident = singles.tile([128, 128], f32)
        make_identity(nc, ident)
        negt = singles.tile([128, S], f32)
        nc.vector.memset(negt, NEG)

        w1sb = singles.tile([H, H], f32)
        w2sb = singles.tile([H, H], f32)
        w3sb = singles.tile([1, H], f32)
        nc.sync.dma_start(out=w1sb, in_=w1[:, :])
        nc.sync.dma_start(out=w2sb, in_=w2[:, :])
        nc.sync.dma_start(out=w3sb, in_=w3[:, :])

        wps = mips_p.tile([128, 256], f32, tag="mips")
        w1T = singles.tile([H, H], f32r)
        nc.tensor.transpose(wps[:, 0:H], w1sb, ident)
        nc.vector.tensor_copy(w1T, wps[:, 0:H])
        wps2 = mips_p.tile([128, 256], f32, tag="mips")
        w2T = singles.tile([H, H], f32r)
        nc.tensor.transpose(wps2[:, 0:H], w2sb, ident)
        nc.vector.tensor_copy(w2T, wps2[:, 0:H])
        wps3 = mips_p.tile([128, 256], f32, tag="mips")
        w3T = singles.tile([H, 1], f32r)
        nc.tensor.transpose(wps3[:, 0:1], w3sb, ident[0:1, 0:1])
        nc.vector.tensor_copy(w3T, wps3[:, 0:1])

        for blk in range(nblk):
            b0 = blk * BB

            # ---- proj_last for this block: PLT[o, b] = W2 @ L_blk.T ----
            lsb = small_p.tile([BB, H], f32, tag="lsb")
            nc.sync.dma_start(out=lsb, in_=l[b0 : b0 + BB, :])
            ltps = mips_p.tile([128, 256], f32, tag="mips")
            nc.tensor.transpose(ltps[:, 0:BB], lsb, ident[0:BB, 0:BB])
            lt = small_p.tile([H, BB], f32r, tag="lt")
            nc.vector.tensor_copy(lt, ltps[:, 0:BB])
            plps = mips_p.tile([128, 256], f32, tag="mips")
            nc.tensor.matmul(plps[:, 0:BB], w2T, lt, start=True, stop=True)
            plt = small_p.tile([H, BB], f32, tag="plt")
            nc.vector.tensor_copy(plt, plps[:, 0:BB])

            mskt = small_p.tile([BB, S], u8, tag="msk")
            nc.gpsimd.dma_start(out=mskt, in_=m[b0 : b0 + BB, :])

            # ---- X loads ----
            xa = xa_p.tile([128, BB, H], f32)
            xb = xb_p.tile([72, BB, H], f32)
            nc.sync.dma_start(
                out=xa, in_=x[b0 : b0 + BB, 0:128, :].rearrange("b s h -> s b h"))
            nc.sync.dma_start(
                out=xb, in_=x[b0 : b0 + BB, 128:200, :].rearrange("b s h -> s b h"))

            # ---- per 2-batch group: transpose -> proj -> tanh -> scores ----
            scps = None
            sc = sc_p.tile([BB, S], f32)
            for g in range(BB // 2):
                i0, i1 = 2 * g, 2 * g + 1
                xtps = xtps_p.tile([128, 400], f32)
                nc.tensor.transpose(xtps[:, 0:128], xa[:, i0, :], ident)
                nc.tensor.transpose(xtps[:, 128:200], xb[:, i0, :], ident[0:72, 0:72])
                nc.tensor.transpose(xtps[:, 200:328], xa[:, i1, :], ident)
                nc.tensor.transpose(xtps[:, 328:400], xb[:, i1, :], ident[0:72, 0:72])
                xt = xt_p.tile([128, 400], f32r)
                if g % 3 == 1:
                    nc.scalar.copy(xt, xtps)
                else:
                    nc.vector.tensor_copy(xt, xtps)

                pjps = pjps_p.tile([128, 400], f32)
                nc.tensor.matmul(pjps, w1T, xt, start=True, stop=True)

                hid = hid_p.tile([128, 400], f32r)
                nc.scalar.activation(hid[:, 0:200], pjps[:, 0:200], Tanh,
                                     bias=plt[:, i0 : i0 + 1])
                nc.scalar.activation(hid[:, 200:400], pjps[:, 200:400], Tanh,
                                     bias=plt[:, i1 : i1 + 1])

                scps = scps_p.tile([1, 400], f32)
                nc.tensor.matmul(scps, w3T, hid, start=True, stop=True)
                stage = stage_p.tile([1, 400], f32)
                if g % 3 == 2:
                    nc.scalar.copy(stage, scps)
                else:
                    nc.vector.tensor_copy(stage, scps)
                nc.gpsimd.dma_start(out=sc[i0 : i0 + 1, :], in_=stage[:, 0:200])
                nc.gpsimd.dma_start(out=sc[i1 : i1 + 1, :], in_=stage[:, 200:400])

            # ---- masked softmax over S (rows = batches) ----
            nc.vector.copy_predicated(sc, mskt, negt[0:BB, :])
            negmax = small_p.tile([BB, 1], f32, tag="negmax")
            nc.vector.tensor_reduce(negmax, sc, mybir.AxisListType.X,
                                    mybir.AluOpType.max, negate=True)
            pb = sc_p.tile([BB, S], f32, tag="pb")
            zt = small_p.tile([BB, 1], f32, tag="zt")
            nc.scalar.activation(pb, sc, Exp, bias=negmax, accum_out=zt)
            rz = small_p.tile([BB, 1], f32, tag="rz")
            nc.vector.reciprocal(rz, zt)
            attn = sc_p.tile([BB, S], f32, tag="attn")
            nc.vector.tensor_scalar_mul(attn, pb, rz)

            # ---- transpose attn -> columns ----
            atps = mips_p.tile([128, 256], f32, tag="mips")
            nc.tensor.transpose(atps[:, 0:BB], attn[:, 0:128], ident[0:BB, 0:BB])
            nc.tensor.transpose(atps[0:72, BB : BB + BB], attn[:, 128:200],
                                ident[0:BB, 0:BB])
            attT = small_p.tile([128, 2 * BB], f32, tag="attT")
            nc.vector.tensor_copy(attT[:, 0:BB], atps[:, 0:BB])
            nc.vector.tensor_copy(attT[0:72, BB : 2 * BB], atps[0:72, BB : 2 * BB])

            # ---- final weighted sum: outT[h, b] = sum_s attn[s,b] * X[s,h] ----
            outps = ops_p.tile([128, 4 * BB], f32)
            for i in range(BB):
                ca = attT[:, i : i + 1]
                cb = attT[0:72, BB + i : BB + i + 1]
                nc.tensor.matmul(outps[:, 4 * i : 4 * i + 1], xa[:, i, :], ca,
                                 start=True, stop=False)
                nc.tensor.matmul(outps[:, 4 * i : 4 * i + 1], xb[:, i, :], cb,
                                 start=False, stop=True)

            outT4 = o_p.tile([128, 4 * BB], f32, tag="outT4")
            nc.vector.tensor_copy(outT4, outps)
            outT = o_p.tile([128, BB], f32, tag="outT")
            nc.vector.tensor_copy(outT, outT4[:, 0 : 4 * BB : 4])
            onps = mips_p.tile([128, 256], f32, tag="mips")
            nc.tensor.transpose(onps[0:BB, 0:128], outT, ident)
            onat = o_p.tile([BB, H], f32, tag="onat")
            nc.vector.tensor_copy(onat, onps[0:BB, 0:128])
            nc.gpsimd.dma_start(out=out[b0 : b0 + BB, :], in_=onat)

    nc.finalize()
    return nc


def _get_nc(nblk=NBLK):
    if nblk not in _cache:
        _cache[nblk] = _build(nblk)
    return _cache[nblk]


def _in_maps(all_memory, last_memory, mask, W1, W2, W3_w, nblk=NBLK):
    ms = np.ascontiguousarray(mask).view(np.uint8)
    lm = np.ascontiguousarray(last_memory[:, 0, :])
    maps = []
    for c in range(NCORES):
        s0 = c * BC
        maps.append({
            "x": np.ascontiguousarray(all_memory[s0 : s0 + BC]),
            "l": np.ascontiguousarray(lm[s0 : s0 + BC]),
            "m": np.ascontiguousarray(ms[s0 : s0 + BC]),
            "w1": np.ascontiguousarray(W1),
            "w2": np.ascontiguousarray(W2),
            "w3": np.ascontiguousarray(W3_w),
        })
    return maps


def run(all_memory, last_memory, mask, W1, W2, W3_w, W3_b=None, trace=False,
        nblk=NBLK):
    from concourse.bass_utils import run_bass_kernel_spmd
    nc = _get_nc(nblk)
    maps = _in_maps(all_memory, last_memory, mask, W1, W2, W3_w, nblk)
    res = run_bass_kernel_spmd(nc, maps, core_ids=list(range(NCORES)),
                               trace=trace)
    full = np.concatenate([r["out"] for r in res.results], axis=0)
    return full.astype(np.float32), res


def kernel(all_memory, last_memory, mask, W1, W2, W3_w, W3_b):
    # W3_b shifts every score equally; softmax is shift-invariant, so it
    # cancels (and it is zeros in setup_inputs).
    full, _ = run(all_memory, last_memory, mask, W1, W2, W3_w)
    return full

